# revision 44
# baseline (speedup 1.0000x reference)
"""Multi-head attention (B=2, S=2048, E=1024, H=16, d_h=64, causal, fp32)
on 8 Trainium2 NeuronCores.

Sharding: tensor-parallel over heads (2 heads/core) for QKV projections and
attention; small AllToAll of the concatenated head outputs (2MB/core); then
sequence-parallel output projection (each core computes 256 output rows per
batch). Matmuls run in float32r (fp32 storage, ~1.6e-4 matmul relerr, 4x the
fp32 rate).

Layouts are fully transposed to avoid per-element transposes:
  x^T [e, q] via PE transpose -> Q^T/K^T/V^T per head [64 d, 2048 q]
  scores S^T [t, q] (stationary = K^T slice, moving = Q^T)
  exp on ACT; causal diagonal zeroed by gpsimd affine_select after exp
  AV with stationary [ones | V_chunk] [128, 65] -> psum row 0 = softmax
  denominator (fused normalizer); DVE reciprocal + gpsimd partition_broadcast
  + DVE multiply normalize into C^T
  W_O with stationary C^T chunks -> output directly in [q, e] layout.

Hardware constraints honored (found empirically):
  - matmul operands must share base_partition and base 64 crashes: all matmul
    operands live at partition base 0 (or base 1 uniformly for the AV
    normalize, which is DVE-only)
  - DVE cannot shift partitions (silently wrong); ACT can: projection head-1
    splits (psum rows 64:128 -> sbuf rows 0:64) go through nc.scalar.copy
  - fp32r matmul inputs must be produced as float32r (copy/DMA-cast rounds)
"""

import numpy as np

import concourse.bacc as bacc
import concourse.mybir as mybir
import concourse.tile as tile
from concourse.bass_utils import run_bass_kernel_spmd
from concourse.masks import make_identity

import os

F32 = mybir.dt.float32
F32R = mybir.dt.float32r
BF16 = mybir.dt.bfloat16
AF = mybir.ActivationFunctionType
# matmul dtype: f32r (safe, ~3e-4 rel err) or bf16 (faster, ~few e-3)
DT = {"f32r": F32R, "bf16": BF16}[os.environ.get("KDT", "bf16")]
DT_X = F32  # x staging dtype: f32 via sync HWDGE (no casting DMAs needed)
# attention dtype (Q/K/V/P tiles): bf16 doubles the scores/AV matmul rate
DT_A = {"f32r": F32R, "bf16": BF16}[os.environ.get("KATT", "bf16")]
DT_AT = DT_X if DT_A is F32R else BF16  # V-transpose dtype

N_CORES = 8
B, S, E = 2, 2048, 1024
H, DH = 16, 64
HPC = H // N_CORES  # heads per core = 2
QS = S // N_CORES  # output q rows per core per batch = 256
SCALE = 1.0 / 8.0  # 1/sqrt(DH)

_NC_CACHE = []


def build_nc():
    nc = bacc.Bacc("TRN2", target_bir_lowering=False, debug=False, num_devices=N_CORES)

    x_d = nc.dram_tensor("x", [B, S, E], F32, kind="ExternalInput").ap()
    wq_d = nc.dram_tensor("wq", [E, HPC * DH], F32, kind="ExternalInput").ap()
    wk_d = nc.dram_tensor("wk", [E, HPC * DH], F32, kind="ExternalInput").ap()
    wv_d = nc.dram_tensor("wv", [E, HPC * DH], F32, kind="ExternalInput").ap()
    wo_d = nc.dram_tensor("wo", [E, E], F32, kind="ExternalInput").ap()
    out_d = nc.dram_tensor("out", [B, QS, E], F32, kind="ExternalOutput").ap()

    with tile.TileContext(nc, trace_sim=False) as tc:
        with (
            tc.tile_pool(name="const", bufs=1) as constp,
            tc.tile_pool(name="wpool", bufs=1) as wpool,
            tc.tile_pool(name="xin", bufs=2) as xin,
            tc.tile_pool(name="wop", bufs=1) as wop,
            tc.tile_pool(name="xtp", bufs=2) as xtp,
            tc.tile_pool(name="qkv", bufs=2 if DT_A is BF16 else 1) as qkvp,
            tc.tile_pool(name="pt", bufs=6) as ptp,
            tc.tile_pool(name="ct", bufs=1) as ctp,
            tc.tile_pool(name="norm", bufs=2) as normp,
            tc.tile_pool(name="cg", bufs=1) as cgp,
            tc.tile_pool(name="osb", bufs=2) as osbp,
            tc.tile_pool(name="psb", bufs=3, space="PSUM") as psb,  # [128,1024] x3 = 6 banks
            tc.tile_pool(name="psm", bufs=2, space="PSUM") as psm,  # [128,512] x2 = 2 banks
            tc.tile_pool(name="dram", bufs=4, space="DRAM") as dramp,
        ):
            ident = constp.tile([128, 128], DT_X, tag="ident")
            make_identity(nc, ident[:])
            if DT_AT is DT_X:
                identb = ident
            else:
                identb = constp.tile([128, 128], DT_AT, tag="identb")
                make_identity(nc, identb[:])
            # static AV stationary tiles [128 t, 16 tc, 1 ones | 64 V]; the
            # ones column FIRST puts the softmax denominator in av row 0, so
            # the normalize broadcast reads partition 0 (the only partition
            # gpsimd partition_broadcast can source); C rows sit at base 1
            # and the a2a staging DMA shifts them back to base 0 for free
            vst_static = []
            for i in range(2):
                t = constp.tile([128, 16, 65], DT_A, tag=f"vstS{i}", name=f"vstS{i}")
                nc.gpsimd.memset(t[:, :, 0:1], 1.0)
                vst_static.append(t)
            # causal diagonal mask: 0 where q_rel >= t_rel else -8000
            # (added to raw scores; exp(scale*(s-8000)) == 0)
            mtri = constp.tile([128, 128], F32, tag="mtri")
            nc.gpsimd.memset(mtri[:], 0.0)
            nc.gpsimd.affine_select(
                out=mtri[:], in_=mtri[:],
                compare_op=mybir.AluOpType.is_ge, fill=-8000.0,
                base=0, pattern=[[1, 128]], channel_multiplier=-1,
            )

            # batch-0 qg0 x tiles first in the sync HWDGE ring so the PE can
            # start transposing as early as possible (x always staged f32 via
            # sync: nothing in that ring ever waits on a collective)
            preloaded = {}
            for qi in range(4):
                t = xin.tile([128, 1024], DT_X, tag=f"xin{qi}")
                nc.sync.dma_start(out=t[:], in_=x_d[0, qi * 128:(qi + 1) * 128, :])
                preloaded[0, qi] = t

            # PE p-state warmup: ~3us of throwaway transposes on the identity
            # while the first x/weight DMAs are in flight, so phase A starts
            # at full clock
            wup = psb.tile([128, 1024], F32, tag="big", name="wup")
            for k in range(28):
                nc.tensor.transpose(
                    wup[:, (k % 8) * 128:(k % 8) * 128 + 128], ident[:], ident[:]
                )

            copy_flip = [0]

            def copy_balanced(dst, src):
                # alternate psum->sbuf evictions between DVE and ACT
                if copy_flip[0] % 2 == 0:
                    nc.vector.tensor_copy(dst, src)
                else:
                    nc.scalar.copy(dst, src)
                copy_flip[0] += 1

            # weight pair tiles [128 e-chunk, 128 (2 heads x 64)]: f32 via the
            # fast sync HWDGE ring + engine cast (the gpsimd software-DGE
            # cast path is ~2.5us per tile and would gate the first
            # projection matmuls by ~25us)
            wtiles = {}
            wstg = {}
            # scalar HWDGE ring: parallel to the x loads on sync (the ring is
            # otherwise only used for tail cg gathers)
            for name, wd in (("q", wq_d), ("k", wk_d), ("v", wv_d)):
                for ec in range(8):
                    s = wpool.tile([128, 128], F32, tag=f"ws{name}{ec}", name=f"ws{name}{ec}")
                    nc.scalar.dma_start(out=s[:], in_=wd[ec * 128:(ec + 1) * 128, :])
                    wstg[name, ec] = s
            for name in ("q", "k", "v"):
                for ec in range(8):
                    t = wpool.tile([128, 128], DT, tag=f"w{name}{ec}", name=f"w{name}{ec}")
                    copy_balanced(t[:], wstg[name, ec][:])
                    wtiles[name, ec] = t
            # W_O as 16 per-head [64, 1024] tiles: lets the E projection
            # accumulate even heads (first collective of the batch) before
            # the odd heads' collective has landed
            wo_tiles = []
            for hd in range(16):
                t = wop.tile([64, 1024], DT, tag=f"wo{hd}", name=f"wo{hd}")
                nc.gpsimd.dma_start(out=t[:], in_=wo_d[hd * 64:(hd + 1) * 64, :])
                wo_tiles.append(t)

            def ab_chunks(b):
                """Phase A+B for batch b as a list of ~0.9us PE chunk
                closures: pop them back-to-back for a standalone phase, or
                one per attention strip round as PE filler (keeps the PE
                p-state at full clock while ACT paces the softmax).

                dve_only evictions keep filler work off the ACT engine, which
                is saturated with exps while filler runs."""
                dve_only = b > 0
                qkv = {}
                for name in ("q", "k", "v"):
                    for h in range(HPC):
                        qkv[name, h] = qkvp.tile(
                            [64, S], DT_A, tag=f"{name}h{h}", name=f"{name}h{h}"
                        )
                xts_box, xtb_box, proj_box = {}, {}, {}

                def dma_chunk(qg):
                    def go():
                        xts = []
                        for qi in range(4):
                            xt = preloaded.pop((b, qg * 4 + qi), None)
                            if xt is None:
                                xt = xin.tile([128, 1024], DT_X, tag=f"xin{qi}")
                                nc.sync.dma_start(
                                    out=xt[:],
                                    in_=x_d[b, (qg * 4 + qi) * 128:(qg * 4 + qi + 1) * 128, :],
                                )
                            xts.append(xt)
                        xts_box[qg] = xts
                    return go

                def t_chunk(qg, ecp):
                    def go():
                        xts = xts_box[qg]
                        ps = psb.tile([128, 1024], F32, tag="big", name="xtps")
                        for hlf in range(2):
                            ec = 2 * ecp + hlf
                            for qi in range(4):
                                nc.tensor.transpose(
                                    ps[:, hlf * 512 + qi * 128: hlf * 512 + (qi + 1) * 128],
                                    xts[qi][:, ec * 128:(ec + 1) * 128],
                                    ident[:],
                                )
                        xt2 = xtp.tile([128, 1024], DT, tag=f"xtb{ecp}")
                        if dve_only:
                            nc.vector.tensor_copy(xt2[:], ps[:])
                        else:
                            copy_balanced(xt2[:], ps[:])
                        xtb_box.setdefault(qg, {})[ecp] = xt2
                    return go

                def m_chunk(qg, name, hf):
                    def go():
                        xtb = xtb_box[qg]
                        if hf == 0:
                            ps = psm.tile([128, 512], F32, tag="mm", name=f"pj{name}")
                            proj_box[qg, name] = ps
                        else:
                            ps = proj_box.pop((qg, name))
                        for ec in range(4 * hf, 4 * hf + 4):
                            nc.tensor.matmul(
                                ps[:],
                                wtiles[name, ec][:],
                                xtb[ec // 2][:, (ec % 2) * 512:(ec % 2) * 512 + 512],
                                start=(ec == 0),
                                stop=(ec == 7),
                            )
                        if hf == 1:
                            sl = slice(qg * 512, qg * 512 + 512)
                            nc.vector.tensor_copy(qkv[name, 0][:, sl], ps[0:64, :])
                            # head-1 rows must shift partitions: ACT only
                            nc.scalar.copy(qkv[name, 1][:, sl], ps[64:128, :])
                    return go

                chunks = []
                for qg in range(4):
                    if qg == 0:
                        chunks.append(dma_chunk(0))
                    if qg + 1 < 4:
                        # prefetch the next q-group's x one group ahead
                        chunks.append(dma_chunk(qg + 1))
                    # interleave transposes with projections: spreads the
                    # DVE/ACT eviction load so no chunk waits on a cast
                    chunks.append(t_chunk(qg, 0))
                    chunks.append(t_chunk(qg, 1))
                    chunks.append(m_chunk(qg, "q", 0))
                    chunks.append(t_chunk(qg, 2))
                    chunks.append(m_chunk(qg, "k", 0))
                    chunks.append(t_chunk(qg, 3))
                    chunks.append(m_chunk(qg, "q", 1))
                    chunks.append(m_chunk(qg, "k", 1))
                    chunks.append(m_chunk(qg, "v", 0))
                    chunks.append(m_chunk(qg, "v", 1))
                return qkv, chunks

            def emit_attention(b, qkv, filler, fill_start=0, fill_every=1):
                rnd = [0]
                # ---- Phase C: attention per head ---------------------------
                # S/A software pipeline: scores for strip t+LAG are emitted
                # before the AV matmuls for strip t, so the PE never waits on
                # the ACT exp of the strip it is about to consume.  Each
                # q-block's accumulator is normalized (and its a2a shards
                # shipped) as soon as its last AV lands, spreading DVE work
                # and freeing av psum slots early.
                LAG = 3
                # ct rows 1:65 hold C^T (uniform base-1 for the DVE normalize;
                # the staging DMA shifts back to base 0 for free)
                ct = [ctp.tile([65, S], DT, tag=f"ct{h}", name=f"ct{h}") for h in range(HPC)]
                a2a_outs = []
                for h in range(HPC):
                    vh = qkv["v", h]
                    vstb = vst_static[h % 2]
                    # V^T: 16 transposes packed into one psum tile, one
                    # strided DVE evict into the [128, 16, 65] stat tile
                    # (ones column 0 preset at build time)
                    vps = psb.tile([128, 16, 64], DT_AT, tag="big", name="vps")
                    for tcx in range(16):
                        vslice = vh[:, tcx * 128:(tcx + 1) * 128]
                        if DT_A is F32R:
                            vslice = vslice.bitcast(DT_X)
                        nc.tensor.transpose(
                            vps[:, tcx, :],
                            vslice,
                            identb[0:64, 0:64],
                        )
                    nc.vector.tensor_copy(vstb[:, :, 1:65], vps[:, :, :])

                    a2a_in = dramp.tile([8, 64, QS], DT, tag=f"a2a_in{h}")
                    a2a_out = dramp.tile([8, 64, QS], DT, tag=f"a2a_out{h}")
                    kh, qh = qkv["k", h], qkv["q", h]
                    av = {}
                    pend = []

                    def normalize_qb(qb):
                        half_, qbr_ = qb // 2, qb % 2
                        a = av[half_, qbr_]
                        # denominator row 0 psum->sbuf on DVE (0->0: no
                        # partition shift), broadcast on gpsimd (reads
                        # partition 0 only), reciprocal+multiply on DVE at
                        # uniform base 1: no ACT involvement (ACT is
                        # saturated with exps here)
                        dsb = normp.tile([1, 512], F32, tag="dsb")
                        nc.vector.tensor_copy(dsb[:], a[0:1, :])
                        bc = normp.tile([65, 512], F32, tag="bc")
                        nc.gpsimd.partition_broadcast(bc[:], dsb[:])
                        bcr = normp.tile([65, 512], F32, tag="bcr")
                        nc.vector.reciprocal_approx_fast(bcr[:], bc[:])
                        # multiply straight out of psum over all 65 rows at
                        # base 0 (engines reject base-1 access; row 0 becomes
                        # a harmless den*recip(den)); frees the av slot
                        nc.vector.tensor_mul(
                            ct[h][0:65, qb * 512:qb * 512 + 512],
                            a[:],
                            bcr[:],
                        )
                        # ship this q-block's two a2a shards immediately
                        # (sync HWDGE: keeps that ring free of
                        # collective-dependent traffic)
                        for j in (2 * qb, 2 * qb + 1):
                            nc.sync.dma_start(
                                out=a2a_in[j],
                                in_=ct[h][1:65, j * QS:(j + 1) * QS],
                            )

                    def flush_av():
                        half_, tcx_, pt_ = pend.pop(0)
                        qbase_ = half_ * 1024
                        t0_ = tcx_ * 128
                        for qbr in range(2):
                            qb = 2 * half_ + qbr
                            if qb * 512 + 512 <= t0_:
                                continue
                            m_lo = max(t0_, qb * 512)
                            nc.tensor.matmul(
                                av[half_, qbr][:, m_lo - qb * 512:512],
                                vstb[:, tcx_, :],
                                pt_[:, m_lo - qbase_:qb * 512 + 512 - qbase_],
                                start=(tcx_ == 0),
                                stop=(tcx_ == (qb + 1) * 4 - 1),
                            )
                            if tcx_ == (qb + 1) * 4 - 1:
                                normalize_qb(qb)

                    for half in range(2):
                        qbase = half * 1024
                        n_tc = 8 * (half + 1)
                        for qbr in range(2):
                            av[half, qbr] = psm.tile(
                                [65, 512], F32, tag="mm", name=f"av{half}{qbr}"
                            )
                        for tcx in range(n_tc):
                            t0 = tcx * 128
                            q_lo = max(t0, qbase)
                            strip = psb.tile([128, 1024], F32, tag="big")
                            # scores into strip (columns relative to qbase)
                            lo_rel = q_lo - qbase
                            segs = []
                            if lo_rel < 512:
                                segs.append((lo_rel, 512))
                                segs.append((512, 1024))
                            else:
                                segs.append((lo_rel, 1024))
                            for s0, s1 in segs:
                                nc.tensor.matmul(
                                    strip[:, s0:s1],
                                    kh[:, t0:t0 + 128],
                                    qh[:, qbase + s0:qbase + s1],
                                    start=True,
                                    stop=True,
                                )
                            if t0 >= qbase:
                                # causal triangle: add -8000 where q < t
                                nc.vector.tensor_add(
                                    strip[:, lo_rel:lo_rel + 128],
                                    strip[:, lo_rel:lo_rel + 128],
                                    mtri[:],
                                )
                            pt = ptp.tile([128, 1024], DT_A, tag="pt")
                            nc.scalar.activation(
                                pt[:, lo_rel:1024],
                                strip[:, lo_rel:1024],
                                AF.Exp,
                                scale=SCALE,
                            )
                            pend.append((half, tcx, pt))
                            if len(pend) > LAG:
                                flush_av()
                            # next-phase PE chunks per strip round: the PE
                            # always has ready work queued while ACT paces the
                            # softmax, so its p-state stays at full clock
                            r = rnd[0] = rnd[0] + 1
                            if filler and r >= fill_start and (r - fill_start) % fill_every == 0:
                                filler.pop(0)()
                    while pend:
                        flush_av()

                    nc.gpsimd.collective_compute(
                        "AllToAll",
                        mybir.AluOpType.bypass,
                        replica_groups=[list(range(N_CORES))],
                        ins=[a2a_in[:].opt()],
                        outs=[a2a_out[:].opt()],
                    )
                    a2a_outs.append(a2a_out)

                # cg gathers wait on collective completion. A DMA dispatch
                # blocks its issuing ENGINE on those semaphores, so they must
                # not share an engine with anything latency-critical. Even
                # heads (h0 collective, lands mid-attention) gather on sync;
                # odd heads of the last batch on scalar (tail only).
                cgO_eng = nc.sync if b == 0 else nc.scalar
                cgE, cgO = [], []
                for i in range(8):
                    t = cgp.tile([64, QS], DT, tag=f"cgE{i}", name=f"cgE{i}")
                    nc.sync.dma_start(out=t[:], in_=a2a_outs[0][i])
                    cgE.append(t)
                for i in range(8):
                    t = cgp.tile([64, QS], DT, tag=f"cgO{i}", name=f"cgO{i}")
                    cgO_eng.dma_start(out=t[:], in_=a2a_outs[1][i])
                    cgO.append(t)

                return cgE, cgO

            def e_chunks(b, cg):
                # ---- Phase E: output projection, head-parity split --------
                # even-head passes depend only on the batch's FIRST
                # collective; only the odd-head passes wait for the last one
                cgE, cgO = cg
                ps_box = {}

                def e_chunk(qt, par):
                    def go():
                        if par == 0:
                            ps = psb.tile([128, 1024], F32, tag="big", name="eps")
                            ps_box[qt] = ps
                            srcs = cgE
                        else:
                            ps = ps_box.pop(qt)
                            srcs = cgO
                        for i in range(8):
                            for oh in range(2):
                                nc.tensor.matmul(
                                    ps[:, oh * 512:(oh + 1) * 512],
                                    srcs[i][:, qt * 128:(qt + 1) * 128],
                                    wo_tiles[2 * i + par][:, oh * 512:(oh + 1) * 512],
                                    start=(par == 0 and i == 0),
                                    stop=(par == 1 and i == 7),
                                )
                        if par == 1:
                            # all-DVE eviction + sync-ring store: the scalar
                            # engine may still be blocked on cg dispatches
                            osb = osbp.tile([128, 1024], F32, tag="osb")
                            nc.vector.tensor_copy(osb[:], ps[:])
                            nc.sync.dma_start(
                                out=out_d[b, qt * 128:(qt + 1) * 128, :], in_=osb[:]
                            )
                    return go

                return [e_chunk(0, 0), e_chunk(1, 0), e_chunk(0, 1), e_chunk(1, 1)]

            # Schedule: AB(0) standalone; attention(0) consumes AB(1)'s
            # chunks as per-round PE filler; leftovers drain before
            # attention(1); E(0) runs while the last collective is in
            # flight, then E(1) trails it.
            qkv0, chunks0 = ab_chunks(0)
            for c in chunks0:
                c()
            qkv1, chunks1 = ab_chunks(1)
            cg0 = emit_attention(0, qkv0, filler=chunks1)
            while chunks1:
                chunks1.pop(0)()
            # E(0) fills attention(1): even-head passes (rounds 8,16) need
            # only cc(0,h0) (done ~80us earlier); odd passes (rounds 24,32)
            # need cc(0,h1) (~55us cushion)
            ec0 = e_chunks(0, cg0)
            cg1 = emit_attention(1, qkv1, filler=ec0, fill_start=8, fill_every=8)
            while ec0:
                ec0.pop(0)()
            for c in e_chunks(1, cg1):
                c()

    nc.compile()
    return nc


def _get_nc():
    if not _NC_CACHE:
        _NC_CACHE.append(build_nc())
    return _NC_CACHE[0]


def run(inputs, trace=False, trace_cores=None):
    nc = _get_nc()
    x = np.ascontiguousarray(np.asarray(inputs["x"], np.float32))
    Wq = np.asarray(inputs["Wq"], np.float32)
    Wk = np.asarray(inputs["Wk"], np.float32)
    Wv = np.asarray(inputs["Wv"], np.float32)
    W_O = np.ascontiguousarray(np.asarray(inputs["W_O"], np.float32))

    in_maps = []
    for j in range(N_CORES):
        h0 = HPC * j
        in_maps.append(
            {
                "x": x,
                "wq": np.ascontiguousarray(
                    np.concatenate([Wq[h0 + i] for i in range(HPC)], axis=1)
                ),
                "wk": np.ascontiguousarray(
                    np.concatenate([Wk[h0 + i] for i in range(HPC)], axis=1)
                ),
                "wv": np.ascontiguousarray(
                    np.concatenate([Wv[h0 + i] for i in range(HPC)], axis=1)
                ),
                "wo": W_O,
            }
        )
    kwargs = {}
    if trace:
        kwargs["trace"] = True
        if trace_cores is not None:
            kwargs["trace_cores"] = trace_cores
    res = run_bass_kernel_spmd(nc, in_maps, core_ids=list(range(N_CORES)), **kwargs)
    out = np.empty((B, S, E), np.float32)
    for j in range(N_CORES):
        out[:, j * QS:(j + 1) * QS, :] = res.results[j]["out"]
    return out, res


def kernel(**inputs) -> np.ndarray:
    out, _ = run(inputs)
    return out



# revision 47
# speedup vs baseline: 1.0050x; 1.0050x over previous
"""Multi-head attention (B=2, S=2048, E=1024, H=16, d_h=64, causal, fp32)
on 8 Trainium2 NeuronCores.

Sharding: tensor-parallel over heads (2 heads/core) for QKV projections and
attention; small AllToAll of the concatenated head outputs (2MB/core); then
sequence-parallel output projection (each core computes 256 output rows per
batch). Matmuls run in float32r (fp32 storage, ~1.6e-4 matmul relerr, 4x the
fp32 rate).

Layouts are fully transposed to avoid per-element transposes:
  x^T [e, q] via PE transpose -> Q^T/K^T/V^T per head [64 d, 2048 q]
  scores S^T [t, q] (stationary = K^T slice, moving = Q^T)
  exp on ACT; causal diagonal zeroed by gpsimd affine_select after exp
  AV with stationary [ones | V_chunk] [128, 65] -> psum row 0 = softmax
  denominator (fused normalizer); DVE reciprocal + gpsimd partition_broadcast
  + DVE multiply normalize into C^T
  W_O with stationary C^T chunks -> output directly in [q, e] layout.

Hardware constraints honored (found empirically):
  - matmul operands must share base_partition and base 64 crashes: all matmul
    operands live at partition base 0 (or base 1 uniformly for the AV
    normalize, which is DVE-only)
  - DVE cannot shift partitions (silently wrong); ACT can: projection head-1
    splits (psum rows 64:128 -> sbuf rows 0:64) go through nc.scalar.copy
  - fp32r matmul inputs must be produced as float32r (copy/DMA-cast rounds)
"""

import numpy as np

import concourse.bacc as bacc
import concourse.mybir as mybir
import concourse.tile as tile
from concourse.bass_utils import run_bass_kernel_spmd
from concourse.masks import make_identity

import os

F32 = mybir.dt.float32
F32R = mybir.dt.float32r
BF16 = mybir.dt.bfloat16
AF = mybir.ActivationFunctionType
# matmul dtype: f32r (safe, ~3e-4 rel err) or bf16 (faster, ~few e-3)
DT = {"f32r": F32R, "bf16": BF16}[os.environ.get("KDT", "bf16")]
DT_X = F32  # x staging dtype: f32 via sync HWDGE (no casting DMAs needed)
# attention dtype (Q/K/V/P tiles): bf16 doubles the scores/AV matmul rate
DT_A = {"f32r": F32R, "bf16": BF16}[os.environ.get("KATT", "bf16")]
DT_AT = DT_X if DT_A is F32R else BF16  # V-transpose dtype

N_CORES = 8
B, S, E = 2, 2048, 1024
H, DH = 16, 64
HPC = H // N_CORES  # heads per core = 2
QS = S // N_CORES  # output q rows per core per batch = 256
SCALE = 1.0 / 8.0  # 1/sqrt(DH)

_NC_CACHE = []


def build_nc():
    nc = bacc.Bacc("TRN2", target_bir_lowering=False, debug=False, num_devices=N_CORES)

    x_d = nc.dram_tensor("x", [B, S, E], F32, kind="ExternalInput").ap()
    wq_d = nc.dram_tensor("wq", [E, HPC * DH], F32, kind="ExternalInput").ap()
    wk_d = nc.dram_tensor("wk", [E, HPC * DH], F32, kind="ExternalInput").ap()
    wv_d = nc.dram_tensor("wv", [E, HPC * DH], F32, kind="ExternalInput").ap()
    wo_d = nc.dram_tensor("wo", [E, E], F32, kind="ExternalInput").ap()
    out_d = nc.dram_tensor("out", [B, QS, E], F32, kind="ExternalOutput").ap()

    with tile.TileContext(nc, trace_sim=False) as tc:
        with (
            tc.tile_pool(name="const", bufs=1) as constp,
            tc.tile_pool(name="wpool", bufs=1) as wpool,
            tc.tile_pool(name="xin", bufs=2) as xin,
            tc.tile_pool(name="wop", bufs=1) as wop,
            tc.tile_pool(name="xtp", bufs=2) as xtp,
            tc.tile_pool(name="qkv", bufs=2 if DT_A is BF16 else 1) as qkvp,
            tc.tile_pool(name="pt", bufs=6) as ptp,
            tc.tile_pool(name="ct", bufs=1) as ctp,
            tc.tile_pool(name="norm", bufs=2) as normp,
            tc.tile_pool(name="cg", bufs=1) as cgp,
            tc.tile_pool(name="osb", bufs=2) as osbp,
            tc.tile_pool(name="psb", bufs=3, space="PSUM") as psb,  # [128,1024] x3 = 6 banks
            tc.tile_pool(name="psm", bufs=2, space="PSUM") as psm,  # [128,512] x2 = 2 banks
            tc.tile_pool(name="dram", bufs=4, space="DRAM") as dramp,
        ):
            ident = constp.tile([128, 128], DT_X, tag="ident")
            make_identity(nc, ident[:])
            if DT_AT is DT_X:
                identb = ident
            else:
                identb = constp.tile([128, 128], DT_AT, tag="identb")
                make_identity(nc, identb[:])
            # static AV stationary tiles [128 t, 16 tc, 1 ones | 64 V]; the
            # ones column FIRST puts the softmax denominator in av row 0, so
            # the normalize broadcast reads partition 0 (the only partition
            # gpsimd partition_broadcast can source); C rows sit at base 1
            # and the a2a staging DMA shifts them back to base 0 for free
            vst_static = []
            for i in range(2):
                t = constp.tile([128, 16, 65], DT_A, tag=f"vstS{i}", name=f"vstS{i}")
                nc.gpsimd.memset(t[:, :, 0:1], 1.0)
                vst_static.append(t)
            # causal diagonal mask: 0 where q_rel >= t_rel else -8000
            # (added to raw scores; exp(scale*(s-8000)) == 0)
            mtri = constp.tile([128, 128], F32, tag="mtri")
            nc.gpsimd.memset(mtri[:], 0.0)
            nc.gpsimd.affine_select(
                out=mtri[:], in_=mtri[:],
                compare_op=mybir.AluOpType.is_ge, fill=-8000.0,
                base=0, pattern=[[1, 128]], channel_multiplier=-1,
            )

            # batch-0 qg0 x tiles first in the sync HWDGE ring so the PE can
            # start transposing as early as possible (x always staged f32 via
            # sync: nothing in that ring ever waits on a collective)
            preloaded = {}
            for qi in range(4):
                t = xin.tile([128, 1024], DT_X, tag=f"xin{qi}")
                nc.sync.dma_start(out=t[:], in_=x_d[0, qi * 128:(qi + 1) * 128, :])
                preloaded[0, qi] = t

            # PE p-state warmup: ~3us of throwaway transposes on the identity
            # while the first x/weight DMAs are in flight, so phase A starts
            # at full clock
            wup = psb.tile([128, 1024], F32, tag="big", name="wup")
            for k in range(28):
                nc.tensor.transpose(
                    wup[:, (k % 8) * 128:(k % 8) * 128 + 128], ident[:], ident[:]
                )

            copy_flip = [0]

            def copy_balanced(dst, src):
                # alternate psum->sbuf evictions between DVE and ACT
                if copy_flip[0] % 2 == 0:
                    nc.vector.tensor_copy(dst, src)
                else:
                    nc.scalar.copy(dst, src)
                copy_flip[0] += 1

            # weight pair tiles [128 e-chunk, 128 (2 heads x 64)]: f32 via the
            # fast sync HWDGE ring + engine cast (the gpsimd software-DGE
            # cast path is ~2.5us per tile and would gate the first
            # projection matmuls by ~25us)
            wtiles = {}
            wstg = {}
            # scalar HWDGE ring: parallel to the x loads on sync (the ring is
            # otherwise only used for tail cg gathers)
            for name, wd in (("q", wq_d), ("k", wk_d), ("v", wv_d)):
                for ec in range(8):
                    s = wpool.tile([128, 128], F32, tag=f"ws{name}{ec}", name=f"ws{name}{ec}")
                    nc.scalar.dma_start(out=s[:], in_=wd[ec * 128:(ec + 1) * 128, :])
                    wstg[name, ec] = s
            for name in ("q", "k", "v"):
                for ec in range(8):
                    t = wpool.tile([128, 128], DT, tag=f"w{name}{ec}", name=f"w{name}{ec}")
                    copy_balanced(t[:], wstg[name, ec][:])
                    wtiles[name, ec] = t
            wo_tiles = []

            def load_wo():
                # W_O as 16 per-head [64, 1024] tiles: lets the E projection
                # accumulate even heads (first collective of the batch)
                # before the odd heads' collective has landed. Emitted after
                # AB(0) so the slow gpsimd casts don't crowd the startup.
                for hd in range(16):
                    t = wop.tile([64, 1024], DT, tag=f"wo{hd}", name=f"wo{hd}")
                    nc.gpsimd.dma_start(out=t[:], in_=wo_d[hd * 64:(hd + 1) * 64, :])
                    wo_tiles.append(t)

            def ab_chunks(b):
                """Phase A+B for batch b as a list of ~0.9us PE chunk
                closures: pop them back-to-back for a standalone phase, or
                one per attention strip round as PE filler (keeps the PE
                p-state at full clock while ACT paces the softmax).

                dve_only evictions keep filler work off the ACT engine, which
                is saturated with exps while filler runs."""
                dve_only = b > 0
                qkv = {}
                for name in ("q", "k", "v"):
                    for h in range(HPC):
                        qkv[name, h] = qkvp.tile(
                            [64, S], DT_A, tag=f"{name}h{h}", name=f"{name}h{h}"
                        )
                xts_box, xtb_box, proj_box = {}, {}, {}

                def dma_chunk(qg):
                    # odd q-groups ride the gpsimd SWDGE ring (no-cast f32 is
                    # full speed there): halves the per-ring startup backlog
                    eng = nc.sync if qg % 2 == 0 else nc.gpsimd

                    def go():
                        xts = []
                        for qi in range(4):
                            xt = preloaded.pop((b, qg * 4 + qi), None)
                            if xt is None:
                                xt = xin.tile([128, 1024], DT_X, tag=f"xin{qi}")
                                eng.dma_start(
                                    out=xt[:],
                                    in_=x_d[b, (qg * 4 + qi) * 128:(qg * 4 + qi + 1) * 128, :],
                                )
                            xts.append(xt)
                        xts_box[qg] = xts
                    return go

                def t_chunk(qg, ecp):
                    def go():
                        xts = xts_box[qg]
                        ps = psb.tile([128, 1024], F32, tag="big", name="xtps")
                        for hlf in range(2):
                            ec = 2 * ecp + hlf
                            for qi in range(4):
                                nc.tensor.transpose(
                                    ps[:, hlf * 512 + qi * 128: hlf * 512 + (qi + 1) * 128],
                                    xts[qi][:, ec * 128:(ec + 1) * 128],
                                    ident[:],
                                )
                        xt2 = xtp.tile([128, 1024], DT, tag=f"xtb{ecp}")
                        if dve_only:
                            nc.vector.tensor_copy(xt2[:], ps[:])
                        else:
                            copy_balanced(xt2[:], ps[:])
                        xtb_box.setdefault(qg, {})[ecp] = xt2
                    return go

                def m_chunk(qg, name, hf):
                    def go():
                        xtb = xtb_box[qg]
                        if hf == 0:
                            ps = psm.tile([128, 512], F32, tag="mm", name=f"pj{name}")
                            proj_box[qg, name] = ps
                        else:
                            ps = proj_box.pop((qg, name))
                        for ec in range(4 * hf, 4 * hf + 4):
                            nc.tensor.matmul(
                                ps[:],
                                wtiles[name, ec][:],
                                xtb[ec // 2][:, (ec % 2) * 512:(ec % 2) * 512 + 512],
                                start=(ec == 0),
                                stop=(ec == 7),
                            )
                        if hf == 1:
                            sl = slice(qg * 512, qg * 512 + 512)
                            nc.vector.tensor_copy(qkv[name, 0][:, sl], ps[0:64, :])
                            # head-1 rows must shift partitions: ACT only
                            nc.scalar.copy(qkv[name, 1][:, sl], ps[64:128, :])
                    return go

                chunks = []
                for qg in range(4):
                    if qg == 0:
                        chunks.append(dma_chunk(0))
                    if qg + 1 < 4:
                        # prefetch the next q-group's x one group ahead
                        chunks.append(dma_chunk(qg + 1))
                    # interleave transposes with projections: spreads the
                    # DVE/ACT eviction load so no chunk waits on a cast
                    chunks.append(t_chunk(qg, 0))
                    chunks.append(t_chunk(qg, 1))
                    chunks.append(m_chunk(qg, "q", 0))
                    chunks.append(t_chunk(qg, 2))
                    chunks.append(m_chunk(qg, "k", 0))
                    chunks.append(t_chunk(qg, 3))
                    chunks.append(m_chunk(qg, "q", 1))
                    chunks.append(m_chunk(qg, "k", 1))
                    chunks.append(m_chunk(qg, "v", 0))
                    chunks.append(m_chunk(qg, "v", 1))
                return qkv, chunks

            def emit_attention(b, qkv, filler, fill_start=0, fill_every=1):
                rnd = [0]
                # ---- Phase C: attention per head ---------------------------
                # S/A software pipeline: scores for strip t+LAG are emitted
                # before the AV matmuls for strip t, so the PE never waits on
                # the ACT exp of the strip it is about to consume.  Each
                # q-block's accumulator is normalized (and its a2a shards
                # shipped) as soon as its last AV lands, spreading DVE work
                # and freeing av psum slots early.
                LAG = 3
                # ct rows 1:65 hold C^T (uniform base-1 for the DVE normalize;
                # the staging DMA shifts back to base 0 for free)
                ct = [ctp.tile([65, S], DT, tag=f"ct{h}", name=f"ct{h}") for h in range(HPC)]
                a2a_outs = []
                for h in range(HPC):
                    vh = qkv["v", h]
                    vstb = vst_static[h % 2]
                    # V^T: 16 transposes packed into one psum tile, one
                    # strided DVE evict into the [128, 16, 65] stat tile
                    # (ones column 0 preset at build time)
                    vps = psb.tile([128, 16, 64], DT_AT, tag="big", name="vps")
                    for tcx in range(16):
                        vslice = vh[:, tcx * 128:(tcx + 1) * 128]
                        if DT_A is F32R:
                            vslice = vslice.bitcast(DT_X)
                        nc.tensor.transpose(
                            vps[:, tcx, :],
                            vslice,
                            identb[0:64, 0:64],
                        )
                    nc.vector.tensor_copy(vstb[:, :, 1:65], vps[:, :, :])

                    a2a_in = dramp.tile([8, 64, QS], DT, tag=f"a2a_in{h}")
                    a2a_out = dramp.tile([8, 64, QS], DT, tag=f"a2a_out{h}")
                    kh, qh = qkv["k", h], qkv["q", h]
                    av = {}
                    pend = []

                    def normalize_qb(qb):
                        half_, qbr_ = qb // 2, qb % 2
                        a = av[half_, qbr_]
                        # denominator row 0 psum->sbuf on DVE (0->0: no
                        # partition shift), broadcast on gpsimd (reads
                        # partition 0 only), reciprocal+multiply on DVE at
                        # uniform base 1: no ACT involvement (ACT is
                        # saturated with exps here)
                        dsb = normp.tile([1, 512], F32, tag="dsb")
                        nc.vector.tensor_copy(dsb[:], a[0:1, :])
                        bc = normp.tile([65, 512], F32, tag="bc")
                        nc.gpsimd.partition_broadcast(bc[:], dsb[:])
                        bcr = normp.tile([65, 512], F32, tag="bcr")
                        nc.vector.reciprocal_approx_fast(bcr[:], bc[:])
                        # multiply straight out of psum over all 65 rows at
                        # base 0 (engines reject base-1 access; row 0 becomes
                        # a harmless den*recip(den)); frees the av slot
                        nc.vector.tensor_mul(
                            ct[h][0:65, qb * 512:qb * 512 + 512],
                            a[:],
                            bcr[:],
                        )
                        # ship this q-block's two a2a shards immediately
                        # (sync HWDGE: keeps that ring free of
                        # collective-dependent traffic)
                        for j in (2 * qb, 2 * qb + 1):
                            nc.sync.dma_start(
                                out=a2a_in[j],
                                in_=ct[h][1:65, j * QS:(j + 1) * QS],
                            )

                    def flush_av():
                        half_, tcx_, pt_ = pend.pop(0)
                        qbase_ = half_ * 1024
                        t0_ = tcx_ * 128
                        for qbr in range(2):
                            qb = 2 * half_ + qbr
                            if qb * 512 + 512 <= t0_:
                                continue
                            m_lo = max(t0_, qb * 512)
                            nc.tensor.matmul(
                                av[half_, qbr][:, m_lo - qb * 512:512],
                                vstb[:, tcx_, :],
                                pt_[:, m_lo - qbase_:qb * 512 + 512 - qbase_],
                                start=(tcx_ == 0),
                                stop=(tcx_ == (qb + 1) * 4 - 1),
                            )
                            if tcx_ == (qb + 1) * 4 - 1:
                                normalize_qb(qb)

                    for half in range(2):
                        qbase = half * 1024
                        n_tc = 8 * (half + 1)
                        for qbr in range(2):
                            av[half, qbr] = psm.tile(
                                [65, 512], F32, tag="mm", name=f"av{half}{qbr}"
                            )
                        for tcx in range(n_tc):
                            t0 = tcx * 128
                            q_lo = max(t0, qbase)
                            strip = psb.tile([128, 1024], F32, tag="big")
                            # scores into strip (columns relative to qbase)
                            lo_rel = q_lo - qbase
                            segs = []
                            if lo_rel < 512:
                                segs.append((lo_rel, 512))
                                segs.append((512, 1024))
                            else:
                                segs.append((lo_rel, 1024))
                            for s0, s1 in segs:
                                nc.tensor.matmul(
                                    strip[:, s0:s1],
                                    kh[:, t0:t0 + 128],
                                    qh[:, qbase + s0:qbase + s1],
                                    start=True,
                                    stop=True,
                                )
                            if t0 >= qbase:
                                # causal triangle: add -8000 where q < t
                                nc.vector.tensor_add(
                                    strip[:, lo_rel:lo_rel + 128],
                                    strip[:, lo_rel:lo_rel + 128],
                                    mtri[:],
                                )
                            pt = ptp.tile([128, 1024], DT_A, tag="pt")
                            nc.scalar.activation(
                                pt[:, lo_rel:1024],
                                strip[:, lo_rel:1024],
                                AF.Exp,
                                scale=SCALE,
                            )
                            pend.append((half, tcx, pt))
                            if len(pend) > LAG:
                                flush_av()
                            # next-phase PE chunks per strip round: the PE
                            # always has ready work queued while ACT paces the
                            # softmax, so its p-state stays at full clock
                            r = rnd[0] = rnd[0] + 1
                            if filler and r >= fill_start and (r - fill_start) % fill_every == 0:
                                filler.pop(0)()
                    while pend:
                        flush_av()

                    nc.gpsimd.collective_compute(
                        "AllToAll",
                        mybir.AluOpType.bypass,
                        replica_groups=[list(range(N_CORES))],
                        ins=[a2a_in[:].opt()],
                        outs=[a2a_out[:].opt()],
                    )
                    a2a_outs.append(a2a_out)

                # cg gathers wait on collective completion. A DMA dispatch
                # blocks its issuing ENGINE on those semaphores, so they must
                # not share an engine with anything latency-critical. Even
                # heads (h0 collective, lands mid-attention) gather on sync;
                # odd heads of the last batch on scalar (tail only).
                cgO_eng = nc.sync if b == 0 else nc.scalar
                cgE, cgO = [], []
                for i in range(8):
                    t = cgp.tile([64, QS], DT, tag=f"cgE{i}", name=f"cgE{i}")
                    nc.sync.dma_start(out=t[:], in_=a2a_outs[0][i])
                    cgE.append(t)
                for i in range(8):
                    t = cgp.tile([64, QS], DT, tag=f"cgO{i}", name=f"cgO{i}")
                    cgO_eng.dma_start(out=t[:], in_=a2a_outs[1][i])
                    cgO.append(t)

                return cgE, cgO

            def e_chunks(b, cg):
                # ---- Phase E: output projection, head-parity split --------
                # even-head passes depend only on the batch's FIRST
                # collective; only the odd-head passes wait for the last one
                cgE, cgO = cg
                ps_box = {}

                def e_chunk(qt, par):
                    def go():
                        if par == 0:
                            ps = psb.tile([128, 1024], F32, tag="big", name="eps")
                            ps_box[qt] = ps
                            srcs = cgE
                        else:
                            ps = ps_box.pop(qt)
                            srcs = cgO
                        for i in range(8):
                            for oh in range(2):
                                nc.tensor.matmul(
                                    ps[:, oh * 512:(oh + 1) * 512],
                                    srcs[i][:, qt * 128:(qt + 1) * 128],
                                    wo_tiles[2 * i + par][:, oh * 512:(oh + 1) * 512],
                                    start=(par == 0 and i == 0),
                                    stop=(par == 1 and i == 7),
                                )
                        if par == 1:
                            # all-DVE eviction + sync-ring store: the scalar
                            # engine may still be blocked on cg dispatches
                            osb = osbp.tile([128, 1024], F32, tag="osb")
                            nc.vector.tensor_copy(osb[:], ps[:])
                            nc.sync.dma_start(
                                out=out_d[b, qt * 128:(qt + 1) * 128, :], in_=osb[:]
                            )
                    return go

                return [e_chunk(0, 0), e_chunk(1, 0), e_chunk(0, 1), e_chunk(1, 1)]

            # Schedule: AB(0) standalone; attention(0) consumes AB(1)'s
            # chunks as per-round PE filler; leftovers drain before
            # attention(1); E(0) runs while the last collective is in
            # flight, then E(1) trails it.
            qkv0, chunks0 = ab_chunks(0)
            for c in chunks0:
                c()
            load_wo()
            qkv1, chunks1 = ab_chunks(1)
            cg0 = emit_attention(0, qkv0, filler=chunks1)
            while chunks1:
                chunks1.pop(0)()
            # E(0) fills attention(1): even-head passes (rounds 8,16) need
            # only cc(0,h0) (done ~80us earlier); odd passes (rounds 24,32)
            # need cc(0,h1) (~55us cushion)
            ec0 = e_chunks(0, cg0)
            cg1 = emit_attention(1, qkv1, filler=ec0, fill_start=8, fill_every=8)
            while ec0:
                ec0.pop(0)()
            for c in e_chunks(1, cg1):
                c()

    nc.compile()
    return nc


def _get_nc():
    if not _NC_CACHE:
        _NC_CACHE.append(build_nc())
    return _NC_CACHE[0]


def run(inputs, trace=False, trace_cores=None):
    nc = _get_nc()
    x = np.ascontiguousarray(np.asarray(inputs["x"], np.float32))
    Wq = np.asarray(inputs["Wq"], np.float32)
    Wk = np.asarray(inputs["Wk"], np.float32)
    Wv = np.asarray(inputs["Wv"], np.float32)
    W_O = np.ascontiguousarray(np.asarray(inputs["W_O"], np.float32))

    in_maps = []
    for j in range(N_CORES):
        h0 = HPC * j
        in_maps.append(
            {
                "x": x,
                "wq": np.ascontiguousarray(
                    np.concatenate([Wq[h0 + i] for i in range(HPC)], axis=1)
                ),
                "wk": np.ascontiguousarray(
                    np.concatenate([Wk[h0 + i] for i in range(HPC)], axis=1)
                ),
                "wv": np.ascontiguousarray(
                    np.concatenate([Wv[h0 + i] for i in range(HPC)], axis=1)
                ),
                "wo": W_O,
            }
        )
    kwargs = {}
    if trace:
        kwargs["trace"] = True
        if trace_cores is not None:
            kwargs["trace_cores"] = trace_cores
    res = run_bass_kernel_spmd(nc, in_maps, core_ids=list(range(N_CORES)), **kwargs)
    out = np.empty((B, S, E), np.float32)
    for j in range(N_CORES):
        out[:, j * QS:(j + 1) * QS, :] = res.results[j]["out"]
    return out, res


def kernel(**inputs) -> np.ndarray:
    out, _ = run(inputs)
    return out



# revision 55
# speedup vs baseline: 1.1156x; 1.1100x over previous
"""Multi-head attention (B=2, S=2048, E=1024, H=16, d_h=64, causal, fp32)
on 8 Trainium2 NeuronCores.

Sharding: tensor-parallel over heads (2 heads/core) for QKV projections and
attention; small AllToAll of the concatenated head outputs (2MB/core); then
sequence-parallel output projection (each core computes 256 output rows per
batch). Matmuls run in float32r (fp32 storage, ~1.6e-4 matmul relerr, 4x the
fp32 rate).

Layouts are fully transposed to avoid per-element transposes:
  x^T [e, q] via PE transpose -> Q^T/K^T/V^T per head [64 d, 2048 q]
  scores S^T [t, q] (stationary = K^T slice, moving = Q^T)
  exp on ACT; causal diagonal zeroed by gpsimd affine_select after exp
  AV with stationary [ones | V_chunk] [128, 65] -> psum row 0 = softmax
  denominator (fused normalizer); DVE reciprocal + gpsimd partition_broadcast
  + DVE multiply normalize into C^T
  W_O with stationary C^T chunks -> output directly in [q, e] layout.

Hardware constraints honored (found empirically):
  - matmul operands must share base_partition and base 64 crashes: all matmul
    operands live at partition base 0 (or base 1 uniformly for the AV
    normalize, which is DVE-only)
  - DVE cannot shift partitions (silently wrong); ACT can: projection head-1
    splits (psum rows 64:128 -> sbuf rows 0:64) go through nc.scalar.copy
  - fp32r matmul inputs must be produced as float32r (copy/DMA-cast rounds)
"""

import numpy as np

import concourse.bacc as bacc
import concourse.mybir as mybir
import concourse.tile as tile
from concourse.bass_utils import run_bass_kernel_spmd
from concourse.masks import make_identity

import os

F32 = mybir.dt.float32
F32R = mybir.dt.float32r
BF16 = mybir.dt.bfloat16
AF = mybir.ActivationFunctionType
# matmul dtype: f32r (safe, ~3e-4 rel err) or bf16 (faster, ~few e-3)
DT = {"f32r": F32R, "bf16": BF16}[os.environ.get("KDT", "bf16")]
DT_X = F32  # x staging dtype: f32 via sync HWDGE (no casting DMAs needed)
# attention dtype (Q/K/V/P tiles): bf16 doubles the scores/AV matmul rate
DT_A = {"f32r": F32R, "bf16": BF16}[os.environ.get("KATT", "bf16")]
DT_AT = DT_X if DT_A is F32R else BF16  # V-transpose dtype

N_CORES = 8
B, S, E = 2, 2048, 1024
H, DH = 16, 64
HPC = H // N_CORES  # heads per core = 2
QS = S // N_CORES  # output q rows per core per batch = 256
SCALE = 1.0 / 8.0  # 1/sqrt(DH)

_NC_CACHE = []


def build_nc():
    nc = bacc.Bacc("TRN2", target_bir_lowering=False, debug=False, num_devices=N_CORES)

    x_d = nc.dram_tensor("x", [B, S, E], F32, kind="ExternalInput").ap()
    wq_d = nc.dram_tensor("wq", [E, HPC * DH], F32, kind="ExternalInput").ap()
    wk_d = nc.dram_tensor("wk", [E, HPC * DH], F32, kind="ExternalInput").ap()
    wv_d = nc.dram_tensor("wv", [E, HPC * DH], F32, kind="ExternalInput").ap()
    wo_d = nc.dram_tensor("wo", [E, E], F32, kind="ExternalInput").ap()
    out_d = nc.dram_tensor("out", [B, QS, E], F32, kind="ExternalOutput").ap()

    with tile.TileContext(nc, trace_sim=False) as tc:
        with (
            tc.tile_pool(name="const", bufs=1) as constp,
            tc.tile_pool(name="wpool", bufs=1) as wpool,
            tc.tile_pool(name="xin", bufs=1) as xin,
            tc.tile_pool(name="wop", bufs=1) as wop,
            tc.tile_pool(name="xtp", bufs=2) as xtp,
            tc.tile_pool(name="qkv", bufs=2 if DT_A is BF16 else 1) as qkvp,
            tc.tile_pool(name="pt", bufs=6) as ptp,
            tc.tile_pool(name="ct", bufs=1) as ctp,
            tc.tile_pool(name="norm", bufs=2) as normp,
            tc.tile_pool(name="cg", bufs=1) as cgp,
            tc.tile_pool(name="osb", bufs=2) as osbp,
            tc.tile_pool(name="psb", bufs=3, space="PSUM") as psb,  # [128,1024] x3 = 6 banks
            tc.tile_pool(name="psm", bufs=2, space="PSUM") as psm,  # [128,512] x2 = 2 banks
            tc.tile_pool(name="dram", bufs=4, space="DRAM") as dramp,
        ):
            ident = constp.tile([128, 128], DT_X, tag="ident")
            make_identity(nc, ident[:])
            if DT_AT is DT_X:
                identb = ident
            else:
                identb = constp.tile([128, 128], DT_AT, tag="identb")
                make_identity(nc, identb[:])
            # static AV stationary tiles [128 t, 16 tc, 1 ones | 64 V]; the
            # ones column FIRST puts the softmax denominator in av row 0, so
            # the normalize broadcast reads partition 0 (the only partition
            # gpsimd partition_broadcast can source); C rows sit at base 1
            # and the a2a staging DMA shifts them back to base 0 for free
            vst_static = []
            for i in range(2):
                t = constp.tile([128, 16, 65], DT_A, tag=f"vstS{i}", name=f"vstS{i}")
                nc.gpsimd.memset(t[:, :, 0:1], 1.0)
                vst_static.append(t)
            # causal diagonal mask: 0 where q_rel >= t_rel else -8000
            # (added to raw scores; exp(scale*(s-8000)) == 0)
            mtri = constp.tile([128, 128], F32, tag="mtri")
            nc.gpsimd.memset(mtri[:], 0.0)
            nc.gpsimd.affine_select(
                out=mtri[:], in_=mtri[:],
                compare_op=mybir.AluOpType.is_ge, fill=-8000.0,
                base=0, pattern=[[1, 128]], channel_multiplier=-1,
            )

            # batch-0 qg0 x tile first in the sync HWDGE ring so the PE can
            # start transposing as early as possible (x always staged f32:
            # neither x ring ever waits on a collective). One DMA per
            # q-group: [128 q-in-tile, 4 q-tiles, 1024 e].
            preloaded = {}

            def load_xg(b, qg):
                eng = nc.sync if qg % 2 == 0 else nc.gpsimd
                t = xin.tile([128, 4, 1024], DT_X, tag=f"xg{qg % 2}", name=f"xg{qg % 2}")
                eng.dma_start(
                    out=t[:],
                    in_=x_d[b, qg * 512:(qg + 1) * 512, :].rearrange(
                        "(qi p) c -> p qi c", p=128
                    ),
                )
                return t

            preloaded[0, 0] = load_xg(0, 0)

            # PE p-state warmup: ~3us of throwaway transposes on the identity
            # while the first x/weight DMAs are in flight, so phase A starts
            # at full clock
            wup = psb.tile([128, 1024], F32, tag="big", name="wup")
            for k in range(28):
                nc.tensor.transpose(
                    wup[:, (k % 8) * 128:(k % 8) * 128 + 128], ident[:], ident[:]
                )

            copy_flip = [0]

            def copy_balanced(dst, src):
                # alternate psum->sbuf evictions between DVE and ACT
                if copy_flip[0] % 2 == 0:
                    nc.vector.tensor_copy(dst, src)
                else:
                    nc.scalar.copy(dst, src)
                copy_flip[0] += 1

            # weight tiles [128 e-in-chunk, 8 e-chunks, 128 (2 heads x 64)]:
            # ONE f32 DMA per name on the fast sync HWDGE ring (dispatches
            # cost ~600ns of engine time each, so fewer+bigger wins) + one
            # DVE cast per name. The slow gpsimd cast path would gate the
            # first projection matmuls by ~25us.
            wtiles = {}
            for name, wd in (("q", wq_d), ("k", wk_d), ("v", wv_d)):
                s = wpool.tile([128, 8, 128], F32, tag=f"ws{name}", name=f"ws{name}")
                nc.sync.dma_start(
                    out=s[:], in_=wd[:].rearrange("(ec p) c -> p ec c", p=128)
                )
                t = wpool.tile([128, 8, 128], DT, tag=f"w{name}", name=f"w{name}")
                copy_balanced(t[:], s[:])
                for ec in range(8):
                    wtiles[name, ec] = t[:, ec, :]
            wo_tiles = []

            def load_wo():
                # W_O as 16 per-head [64, 1024] tiles: lets the E projection
                # accumulate even heads (first collective of the batch)
                # before the odd heads' collective has landed. Emitted after
                # AB(0) so the slow gpsimd casts don't crowd the startup.
                for hd in range(16):
                    t = wop.tile([64, 1024], DT, tag=f"wo{hd}", name=f"wo{hd}")
                    nc.gpsimd.dma_start(out=t[:], in_=wo_d[hd * 64:(hd + 1) * 64, :])
                    wo_tiles.append(t)

            def ab_chunks(b):
                """Phase A+B for batch b as a list of ~0.9us PE chunk
                closures: pop them back-to-back for a standalone phase, or
                one per attention strip round as PE filler (keeps the PE
                p-state at full clock while ACT paces the softmax).

                dve_only evictions keep filler work off the ACT engine, which
                is saturated with exps while filler runs."""
                dve_only = b > 0
                qkv = {}
                for name in ("q", "k", "v"):
                    for h in range(HPC):
                        qkv[name, h] = qkvp.tile(
                            [64, S], DT_A, tag=f"{name}h{h}", name=f"{name}h{h}"
                        )
                xts_box, xtb_box, proj_box = {}, {}, {}

                def dma_chunk(qg):
                    # odd q-groups ride the gpsimd SWDGE ring (no-cast f32 is
                    # full speed there): halves the per-ring startup backlog
                    def go():
                        xt = preloaded.pop((b, qg), None)
                        if xt is None:
                            xt = load_xg(b, qg)
                        xts_box[qg] = xt
                    return go

                def t_chunk(qg, ecp):
                    def go():
                        xt4 = xts_box[qg]
                        ps = psb.tile([128, 1024], F32, tag="big", name="xtps")
                        for hlf in range(2):
                            ec = 2 * ecp + hlf
                            for qi in range(4):
                                nc.tensor.transpose(
                                    ps[:, hlf * 512 + qi * 128: hlf * 512 + (qi + 1) * 128],
                                    xt4[:, qi, ec * 128:(ec + 1) * 128],
                                    ident[:],
                                )
                        xt2 = xtp.tile([128, 1024], DT, tag=f"xtb{ecp}")
                        if dve_only:
                            nc.vector.tensor_copy(xt2[:], ps[:])
                        else:
                            copy_balanced(xt2[:], ps[:])
                        xtb_box.setdefault(qg, {})[ecp] = xt2
                    return go

                def m_chunk(qg, name, hf):
                    def go():
                        xtb = xtb_box[qg]
                        if hf == 0:
                            ps = psm.tile([128, 512], F32, tag="mm", name=f"pj{name}")
                            proj_box[qg, name] = ps
                        else:
                            ps = proj_box.pop((qg, name))
                        for ec in range(4 * hf, 4 * hf + 4):
                            nc.tensor.matmul(
                                ps[:],
                                wtiles[name, ec],
                                xtb[ec // 2][:, (ec % 2) * 512:(ec % 2) * 512 + 512],
                                start=(ec == 0),
                                stop=(ec == 7),
                            )
                        if hf == 1:
                            sl = slice(qg * 512, qg * 512 + 512)
                            nc.vector.tensor_copy(qkv[name, 0][:, sl], ps[0:64, :])
                            # head-1 rows must shift partitions: ACT only
                            nc.scalar.copy(qkv[name, 1][:, sl], ps[64:128, :])
                    return go

                chunks = []
                for qg in range(4):
                    if qg == 0:
                        chunks.append(dma_chunk(0))
                    if qg + 1 < 4:
                        # prefetch the next q-group's x one group ahead
                        chunks.append(dma_chunk(qg + 1))
                    # interleave transposes with projections: spreads the
                    # DVE/ACT eviction load so no chunk waits on a cast
                    chunks.append(t_chunk(qg, 0))
                    chunks.append(t_chunk(qg, 1))
                    chunks.append(m_chunk(qg, "q", 0))
                    chunks.append(t_chunk(qg, 2))
                    chunks.append(m_chunk(qg, "k", 0))
                    chunks.append(t_chunk(qg, 3))
                    chunks.append(m_chunk(qg, "q", 1))
                    chunks.append(m_chunk(qg, "k", 1))
                    chunks.append(m_chunk(qg, "v", 0))
                    chunks.append(m_chunk(qg, "v", 1))
                return qkv, chunks

            def emit_attention(b, qkv, filler, fill_start=0, fill_every=1):
                rnd = [0]
                # ---- Phase C: attention per head ---------------------------
                # S/A software pipeline: scores for strip t+LAG are emitted
                # before the AV matmuls for strip t, so the PE never waits on
                # the ACT exp of the strip it is about to consume.  Each
                # q-block's accumulator is normalized (and its a2a shards
                # shipped) as soon as its last AV lands, spreading DVE work
                # and freeing av psum slots early.
                LAG = 3
                # ct rows 1:65 hold C^T (uniform base-1 for the DVE normalize;
                # the staging DMA shifts back to base 0 for free)
                ct = [ctp.tile([65, S], DT, tag=f"ct{h}", name=f"ct{h}") for h in range(HPC)]
                a2a_outs = []
                for h in range(HPC):
                    vh = qkv["v", h]
                    vstb = vst_static[h % 2]
                    # V^T: 16 transposes packed into one psum tile, one
                    # strided DVE evict into the [128, 16, 65] stat tile
                    # (ones column 0 preset at build time)
                    vps = psb.tile([128, 16, 64], DT_AT, tag="big", name="vps")
                    for tcx in range(16):
                        vslice = vh[:, tcx * 128:(tcx + 1) * 128]
                        if DT_A is F32R:
                            vslice = vslice.bitcast(DT_X)
                        nc.tensor.transpose(
                            vps[:, tcx, :],
                            vslice,
                            identb[0:64, 0:64],
                        )
                    nc.vector.tensor_copy(vstb[:, :, 1:65], vps[:, :, :])

                    a2a_in = dramp.tile([8, 64, QS], DT, tag=f"a2a_in{h}")
                    a2a_out = dramp.tile([8, 64, QS], DT, tag=f"a2a_out{h}")
                    kh, qh = qkv["k", h], qkv["q", h]
                    av = {}
                    pend = []

                    def normalize_qb(qb):
                        half_, qbr_ = qb // 2, qb % 2
                        a = av[half_, qbr_]
                        # denominator row 0 psum->sbuf on DVE (0->0: no
                        # partition shift), broadcast on gpsimd (reads
                        # partition 0 only), reciprocal+multiply on DVE at
                        # uniform base 1: no ACT involvement (ACT is
                        # saturated with exps here)
                        dsb = normp.tile([1, 512], F32, tag="dsb")
                        nc.vector.tensor_copy(dsb[:], a[0:1, :])
                        bc = normp.tile([65, 512], F32, tag="bc")
                        nc.gpsimd.partition_broadcast(bc[:], dsb[:])
                        bcr = normp.tile([65, 512], F32, tag="bcr")
                        nc.vector.reciprocal_approx_fast(bcr[:], bc[:])
                        # multiply straight out of psum over all 65 rows at
                        # base 0 (engines reject base-1 access; row 0 becomes
                        # a harmless den*recip(den)); frees the av slot
                        nc.vector.tensor_mul(
                            ct[h][0:65, qb * 512:qb * 512 + 512],
                            a[:],
                            bcr[:],
                        )
                        # ship this q-block's two a2a shards in one DMA
                        # (sync HWDGE: keeps that ring free of
                        # collective-dependent traffic)
                        nc.sync.dma_start(
                            out=a2a_in[2 * qb:2 * qb + 2].rearrange("j p q -> p j q"),
                            in_=ct[h][1:65, qb * 512:(qb + 1) * 512],
                        )

                    def flush_av():
                        half_, tcx_, pt_ = pend.pop(0)
                        qbase_ = half_ * 1024
                        t0_ = tcx_ * 128
                        for qbr in range(2):
                            qb = 2 * half_ + qbr
                            if qb * 512 + 512 <= t0_:
                                continue
                            m_lo = max(t0_, qb * 512)
                            nc.tensor.matmul(
                                av[half_, qbr][:, m_lo - qb * 512:512],
                                vstb[:, tcx_, :],
                                pt_[:, m_lo - qbase_:qb * 512 + 512 - qbase_],
                                start=(tcx_ == 0),
                                stop=(tcx_ == (qb + 1) * 4 - 1),
                            )
                            if tcx_ == (qb + 1) * 4 - 1:
                                normalize_qb(qb)

                    for half in range(2):
                        qbase = half * 1024
                        n_tc = 8 * (half + 1)
                        for qbr in range(2):
                            av[half, qbr] = psm.tile(
                                [65, 512], F32, tag="mm", name=f"av{half}{qbr}"
                            )
                        for tcx in range(n_tc):
                            t0 = tcx * 128
                            q_lo = max(t0, qbase)
                            strip = psb.tile([128, 1024], F32, tag="big")
                            # scores into strip (columns relative to qbase)
                            lo_rel = q_lo - qbase
                            segs = []
                            if lo_rel < 512:
                                segs.append((lo_rel, 512))
                                segs.append((512, 1024))
                            else:
                                segs.append((lo_rel, 1024))
                            for s0, s1 in segs:
                                nc.tensor.matmul(
                                    strip[:, s0:s1],
                                    kh[:, t0:t0 + 128],
                                    qh[:, qbase + s0:qbase + s1],
                                    start=True,
                                    stop=True,
                                )
                            if t0 >= qbase:
                                # causal triangle: add -8000 where q < t
                                nc.vector.tensor_add(
                                    strip[:, lo_rel:lo_rel + 128],
                                    strip[:, lo_rel:lo_rel + 128],
                                    mtri[:],
                                )
                            pt = ptp.tile([128, 1024], DT_A, tag="pt")
                            nc.scalar.activation(
                                pt[:, lo_rel:1024],
                                strip[:, lo_rel:1024],
                                AF.Exp,
                                scale=SCALE,
                            )
                            pend.append((half, tcx, pt))
                            if len(pend) > LAG:
                                flush_av()
                            # next-phase PE chunks per strip round: the PE
                            # always has ready work queued while ACT paces the
                            # softmax, so its p-state stays at full clock
                            r = rnd[0] = rnd[0] + 1
                            if filler and r >= fill_start and (r - fill_start) % fill_every == 0:
                                filler.pop(0)()
                    while pend:
                        flush_av()

                    nc.gpsimd.collective_compute(
                        "AllToAll",
                        mybir.AluOpType.bypass,
                        replica_groups=[list(range(N_CORES))],
                        ins=[a2a_in[:].opt()],
                        outs=[a2a_out[:].opt()],
                    )
                    a2a_outs.append(a2a_out)

                # cg gathers wait on collective completion. A DMA dispatch
                # blocks its issuing ENGINE on those semaphores, so they must
                # not share an engine with anything latency-critical. Even
                # heads (h0 collective, lands mid-attention) gather on sync;
                # odd heads of the last batch on scalar (tail only). One DMA
                # per parity: [64 d, 8 src cores, QS q].
                cgO_eng = nc.sync if b == 0 else nc.scalar
                cgE = cgp.tile([64, 8, QS], DT, tag="cgE", name="cgE")
                nc.sync.dma_start(
                    out=cgE[:], in_=a2a_outs[0][:].rearrange("i p q -> p i q")
                )
                cgO = cgp.tile([64, 8, QS], DT, tag="cgO", name="cgO")
                cgO_eng.dma_start(
                    out=cgO[:], in_=a2a_outs[1][:].rearrange("i p q -> p i q")
                )

                return cgE, cgO

            def e_chunks(b, cg):
                # ---- Phase E: output projection, head-parity split --------
                # even-head passes depend only on the batch's FIRST
                # collective; only the odd-head passes wait for the last one
                cgE, cgO = cg
                ps_box = {}

                def e_chunk(qt, par):
                    def go():
                        if par == 0:
                            ps = psb.tile([128, 1024], F32, tag="big", name="eps")
                            ps_box[qt] = ps
                            srcs = cgE
                        else:
                            ps = ps_box.pop(qt)
                            srcs = cgO
                        for i in range(8):
                            for oh in range(2):
                                nc.tensor.matmul(
                                    ps[:, oh * 512:(oh + 1) * 512],
                                    srcs[:, i, qt * 128:(qt + 1) * 128],
                                    wo_tiles[2 * i + par][:, oh * 512:(oh + 1) * 512],
                                    start=(par == 0 and i == 0),
                                    stop=(par == 1 and i == 7),
                                )
                        if par == 1:
                            # all-DVE eviction + sync-ring store: the scalar
                            # engine may still be blocked on cg dispatches
                            osb = osbp.tile([128, 1024], F32, tag="osb")
                            nc.vector.tensor_copy(osb[:], ps[:])
                            nc.sync.dma_start(
                                out=out_d[b, qt * 128:(qt + 1) * 128, :], in_=osb[:]
                            )
                    return go

                return [e_chunk(0, 0), e_chunk(1, 0), e_chunk(0, 1), e_chunk(1, 1)]

            # Schedule: AB(0) standalone; attention(0) consumes AB(1)'s
            # chunks as per-round PE filler; leftovers drain before
            # attention(1); E(0) runs while the last collective is in
            # flight, then E(1) trails it.
            qkv0, chunks0 = ab_chunks(0)
            for c in chunks0:
                c()
            load_wo()
            qkv1, chunks1 = ab_chunks(1)
            cg0 = emit_attention(0, qkv0, filler=chunks1)
            while chunks1:
                chunks1.pop(0)()
            # E(0) fills attention(1): even-head passes (rounds 8,16) need
            # only cc(0,h0) (done ~80us earlier); odd passes (rounds 24,32)
            # need cc(0,h1) (~55us cushion)
            ec0 = e_chunks(0, cg0)
            cg1 = emit_attention(1, qkv1, filler=ec0, fill_start=8, fill_every=8)
            while ec0:
                ec0.pop(0)()
            for c in e_chunks(1, cg1):
                c()

    nc.compile()
    return nc


def _get_nc():
    if not _NC_CACHE:
        _NC_CACHE.append(build_nc())
    return _NC_CACHE[0]


def run(inputs, trace=False, trace_cores=None):
    nc = _get_nc()
    x = np.ascontiguousarray(np.asarray(inputs["x"], np.float32))
    Wq = np.asarray(inputs["Wq"], np.float32)
    Wk = np.asarray(inputs["Wk"], np.float32)
    Wv = np.asarray(inputs["Wv"], np.float32)
    W_O = np.ascontiguousarray(np.asarray(inputs["W_O"], np.float32))

    in_maps = []
    for j in range(N_CORES):
        h0 = HPC * j
        in_maps.append(
            {
                "x": x,
                "wq": np.ascontiguousarray(
                    np.concatenate([Wq[h0 + i] for i in range(HPC)], axis=1)
                ),
                "wk": np.ascontiguousarray(
                    np.concatenate([Wk[h0 + i] for i in range(HPC)], axis=1)
                ),
                "wv": np.ascontiguousarray(
                    np.concatenate([Wv[h0 + i] for i in range(HPC)], axis=1)
                ),
                "wo": W_O,
            }
        )
    kwargs = {}
    if trace:
        kwargs["trace"] = True
        if trace_cores is not None:
            kwargs["trace_cores"] = trace_cores
    res = run_bass_kernel_spmd(nc, in_maps, core_ids=list(range(N_CORES)), **kwargs)
    out = np.empty((B, S, E), np.float32)
    for j in range(N_CORES):
        out[:, j * QS:(j + 1) * QS, :] = res.results[j]["out"]
    return out, res


def kernel(**inputs) -> np.ndarray:
    out, _ = run(inputs)
    return out

